# revision 1
# baseline (speedup 1.0000x reference)
"""Trainium2 Bass kernel for MultiRelationGNN (2-layer relational message
passing, N=50000 nodes, E=1.6M edges, H=64, R=8 relations) on 8 NeuronCores.

Strategy (edge-parallel by destination shard):
  - Nodes are renumbered (degree-balanced snake) into 8 shards x 49 windows of
    128 slots. Each core owns the edges whose dst falls in its shard.
  - Per layer: every core gathers h[src] rows (augmented [h|1|0..] 512B rows)
    from a replicated DRAM table via dma_gather, scales by w = lam*exp(-beta*t),
    and segment-sums into per-(window, relation) aggregates G via PE matmuls
    against device-built one-hot matrices (edges x combined (rel,dst) columns).
  - Aggregate-transform: h1 = sum_r(G_r @ A_r + c_r*b_r + c_r . (h0 @ B_r)),
    computed per window on PE/DVE; no per-edge second matmul is needed.
  - Tables are rebuilt (PE transposes) and AllGathered between layers.
  - Output: out = sum_l lrelu(h_l @ W_out_l + b_out_l) over own shard,
    assembled host-side through the inverse node permutation.
"""
import numpy as np

NCORES = 8
GRP = 128          # window width (nodes per window)
H = 64
AUG = 65
R = 8
LRELU_SLOPE = 0.01
LAST_EXEC_NS = None


def _wrap16(ix):
    """idx vector (mult of 16) -> [128, n/16] int16, replicated per 16-part group."""
    n = len(ix)
    a = ix.astype(np.int16).reshape(n // 16, 16).T
    return np.tile(a, (8, 1))


def _wrap128(v):
    """value vector (mult of 128) -> [128, n/128] (edge i -> [i%128, i//128])."""
    n = len(v)
    return np.ascontiguousarray(v.reshape(n // 128, 128).T)


def _host_prep(src_slot, dst_slot, rel, tim, nwin, shard, half):
    """Per (core, array): sorted/padded edge streams + one-hot pass ids.

    Chunk c of every window gets a core-invariant list of rel-block bases
    BS[c] (union over cores/windows of blocks hit); pass p matches edges
    whose combined col falls in block BS[c][p].
    """
    core = dst_slot // shard
    halfsel = (src_slot >= half).astype(np.int64)

    percore = []
    wsz = 1
    for k in range(NCORES):
        mk = core == k
        data = [src_slot[mk], dst_slot[mk] % shard, rel[mk], tim[mk], halfsel[mk]]
        for a in (0, 1):
            ma = data[4] == a
            wv = data[1][ma] // GRP
            cnts = np.bincount(wv, minlength=nwin)
            wsz = max(wsz, int(cnts.max()))
        # pre-sort per array
        entry = {}
        for a in (0, 1):
            ma = data[4] == a
            s_a, d_a, r_a, t_a = (data[0][ma], data[1][ma], data[2][ma],
                                  data[3][ma])
            w_a = d_a // GRP
            o = np.lexsort((s_a, r_a, w_a))
            entry[a] = tuple(x[o] for x in (s_a, d_a, r_a, t_a, w_a))
        percore.append(entry)
    WSZ = ((wsz + 127) // 128) * 128
    NCH = WSZ // 128

    # sweep 1: block sets per chunk index (union over cores/arrays/windows)
    bsets = [set() for _ in range(NCH)]
    for k in range(NCORES):
        for a in (0, 1):
            s_a, d_a, r_a, t_a, w_a = percore[k][a]
            wstart = np.searchsorted(w_a, np.arange(nwin + 1))
            for w in range(nwin):
                lo, hi = int(wstart[w]), int(wstart[w + 1])
                rr = r_a[lo:hi]
                for c in range((hi - lo + 127) // 128):
                    blk = np.unique(rr[c * 128:min((c + 1) * 128, hi - lo)])
                    bsets[c].update(int(b) for b in blk)
    BS = [sorted(bs) if bs else [0] for bs in bsets]
    MP = max(len(b) for b in BS)

    inputs = []
    for k in range(NCORES):
        d = {}
        for a in (0, 1):
            s_a, d_a, r_a, t_a, w_a = percore[k][a]
            gi = np.zeros(nwin * WSZ, np.int64)
            tw = np.full(nwin * WSZ, 1e9, np.float32)
            rid = np.full((MP, nwin * WSZ), -1.0, np.float32)
            wstart = np.searchsorted(w_a, np.arange(nwin + 1))
            for w in range(nwin):
                lo, hi = int(wstart[w]), int(wstart[w + 1])
                n = hi - lo
                off = w * WSZ
                gi[off:off + n] = s_a[lo:hi] - a * half
                tw[off:off + n] = t_a[lo:hi]
                cols = (r_a[lo:hi] * GRP + (d_a[lo:hi] % GRP)).astype(np.int64)
                for c in range((n + 127) // 128):
                    clo, chi = c * 128, min((c + 1) * 128, n)
                    ch = cols[clo:chi]
                    blk = ch // GRP
                    for p, b in enumerate(BS[c]):
                        m = blk == b
                        seg = np.full(chi - clo, -1.0, np.float32)
                        seg[m] = ch[m] - b * GRP
                        rid[p, off + clo:off + chi] = seg
            d[f"gi{a}"] = np.stack([_wrap16(gi[w * WSZ:(w + 1) * WSZ])
                                    for w in range(nwin)])
            d[f"tw{a}"] = np.stack([_wrap128(tw[w * WSZ:(w + 1) * WSZ])
                                    for w in range(nwin)])
            for p in range(MP):
                d[f"r{p}_{a}"] = np.stack(
                    [_wrap128(rid[p, w * WSZ:(w + 1) * WSZ])
                     for w in range(nwin)])
        inputs.append(d)
    return WSZ, NCH, BS, MP, inputs


def kernel(x, edge_index, edge_type, edge_time, lambda_sym, beta,
           W_field, b_field, W_rel1, b_rel1, W_rel2, b_rel2,
           W_out0, b_out0, W_out1, b_out1, W_out2, b_out2):
    import concourse.bacc as bacc
    import concourse.tile as tile
    import concourse.bass as bass
    import concourse.mybir as mybir
    from concourse.bass_utils import run_bass_kernel_spmd

    x = np.asarray(x, np.float32)
    edge_index = np.asarray(edge_index)
    edge_type = np.asarray(edge_type)
    edge_time = np.asarray(edge_time, np.float32)
    N, IN_DIM = x.shape
    OUT = np.asarray(W_out0).shape[1]
    lam = float(np.asarray(lambda_sym))
    bet = float(np.asarray(beta))

    # ---- node renumbering: degree-balanced snake over all windows ----
    nwin_total = ((N + NCORES * GRP - 1) // (NCORES * GRP))  # windows per core
    NWIN = nwin_total * NCORES
    SHARD = nwin_total * GRP
    NPAD = SHARD * NCORES
    HALF = NPAD // 2
    assert HALF % 2 == 0 and HALF < 32768

    src, dst = edge_index[0].astype(np.int64), edge_index[1].astype(np.int64)
    deg = np.bincount(dst, minlength=N)
    order_nodes = np.argsort(-deg, kind="stable")
    slot = np.zeros(N, np.int64)
    cnt = np.zeros(NWIN, np.int64)
    for i, v in enumerate(order_nodes):
        rowpos, cyc = i % NWIN, i // NWIN
        g = rowpos if cyc % 2 == 0 else NWIN - 1 - rowpos
        slot[v] = g * GRP + cnt[g]
        cnt[g] += 1
    assert cnt.max() <= GRP
    node_of_slot = np.full(NPAD, -1, np.int64)
    node_of_slot[slot] = np.arange(N)

    s_slot, d_slot = slot[src], slot[dst]
    WSZ, NCH, BS, MP, edge_inputs = _host_prep(
        s_slot, d_slot, edge_type.astype(np.int64), edge_time,
        nwin_total, SHARD, HALF)
    NW = nwin_total

    # ---- weight repacks ----
    W_rel1 = np.asarray(W_rel1, np.float32)
    W_rel2 = np.asarray(W_rel2, np.float32)
    b_rel1 = np.asarray(b_rel1, np.float32)
    b_rel2 = np.asarray(b_rel2, np.float32)

    def aug_pack(Wr, br):
        out = np.zeros((AUG, R * H), np.float32)
        for r in range(R):
            out[:H, r * H:(r + 1) * H] = Wr[r, :H, :]
            out[H, r * H:(r + 1) * H] = br[r]
        return out

    def b_pack(Wr):
        out = np.zeros((H, R * H), np.float32)
        for r in range(R):
            out[:, r * H:(r + 1) * H] = Wr[r, H:, :]
        return out

    waug1_np, waug2_np = aug_pack(W_rel1, b_rel1), aug_pack(W_rel2, b_rel2)
    bmat1_np, bmat2_np = b_pack(W_rel1), b_pack(W_rel2)
    wout_np = np.concatenate([np.asarray(W_out0, np.float32),
                              np.asarray(W_out1, np.float32),
                              np.asarray(W_out2, np.float32)], axis=1)  # [H, 3*OUT]
    bout_np = np.stack([np.asarray(b_out0, np.float32),
                        np.asarray(b_out1, np.float32),
                        np.asarray(b_out2, np.float32)], axis=1)  # [OUT, 3]
    iota_np = np.tile(np.arange(GRP, dtype=np.float32), (128, 1))
    ident_np = np.eye(128, dtype=np.float32)
    sel_np = np.zeros((AUG, H), np.float32)
    sel_np[H, :] = 1.0
    ones_np = np.ones((1, 512), np.float32)
    bfield_np = np.asarray(b_field, np.float32).reshape(1, H)
    wfield_np = np.asarray(W_field, np.float32)

    # per-core xT (padded slots -> zero rows)
    x_slot = np.zeros((NPAD, IN_DIM), np.float32)
    x_slot[slot] = x

    # ---- build program ----
    nc = bacc.Bacc("TRN2", target_bir_lowering=False, debug=False,
                   enable_asserts=False, num_devices=NCORES,
                   num_swdge_queues=4)
    dt = mybir.dt

    def inp(name, shape, d=dt.float32):
        return nc.dram_tensor(name, shape, d, kind="ExternalInput").ap()

    xT_in = inp("xT", [IN_DIM, SHARD])
    gi_in = {a: inp(f"gi{a}", [NW, 128, WSZ // 16], dt.int16) for a in (0, 1)}
    tw_in = {a: inp(f"tw{a}", [NW, 128, NCH]) for a in (0, 1)}
    rid_in = {(p, a): inp(f"r{p}_{a}", [NW, 128, NCH])
              for p in range(MP) for a in (0, 1)}
    wfield_in = inp("wfield", [IN_DIM, H])
    bfield_in = inp("bfield", [1, H])
    waug_in = {1: inp("waug1", [AUG, R * H]), 2: inp("waug2", [AUG, R * H])}
    bmat_in = {1: inp("bmat1", [H, R * H]), 2: inp("bmat2", [H, R * H])}
    wout_in = inp("wout", [H, 3 * OUT])
    bout_in = inp("bout", [OUT, 3])
    iota_in = inp("iota", [128, GRP])
    ident_in = inp("ident", [128, 128])
    sel_in = inp("sel", [AUG, H])
    ones_in = inp("ones", [1, 512])
    outT = nc.dram_tensor("outT", [OUT, SHARD], dt.float32,
                          kind="ExternalOutput").ap()

    # internal DRAM: per-layer tables + AG slabs
    slab = {l: nc.dram_tensor(f"slab{l}", [SHARD, 128], dt.float32,
                              kind="Internal").ap() for l in (0, 1)}
    table = {l: nc.dram_tensor(f"table{l}", [NPAD, 128], dt.float32,
                               kind="Internal", addr_space="Shared").ap()
             for l in (0, 1)}

    LN_LAM = float(np.log(lam))

    with tile.TileContext(nc) as tc:
        with tc.tile_pool(name="const", bufs=1) as cpool:
            wfield_t = cpool.tile([IN_DIM, H], dt.float32)
            bfield_t = cpool.tile([1, H], dt.float32)
            waug1_t = cpool.tile([AUG, R * H], dt.float32)
            waug2_t = cpool.tile([AUG, R * H], dt.float32)
            bmat1_t = cpool.tile([H, R * H], dt.float32)
            bmat2_t = cpool.tile([H, R * H], dt.float32)
            waug_t = {1: waug1_t, 2: waug2_t}
            bmat_t = {1: bmat1_t, 2: bmat2_t}
            wout_t = cpool.tile([H, 3 * OUT], dt.float32)
            bout_t = cpool.tile([OUT, 3], dt.float32)
            iota_t = cpool.tile([128, GRP], dt.float32)
            ident_t = cpool.tile([128, 128], dt.float32)
            sel_t = cpool.tile([AUG, H], dt.float32)
            ones_t = cpool.tile([1, 512], dt.float32)
            lnlam_t = cpool.tile([128, 1], dt.float32)
            onebias_t = cpool.tile([128, 1], dt.float32)
            h0T_t = cpool.tile([H, SHARD], dt.float32)
            h1T_t = cpool.tile([H, SHARD], dt.float32)
            h2T_t = cpool.tile([H, SHARD], dt.float32)
            hT = {0: h0T_t, 1: h1T_t, 2: h2T_t}
            for t_, s_ in [(wfield_t, wfield_in), (bfield_t, bfield_in),
                           (waug_t[1], waug_in[1]), (waug_t[2], waug_in[2]),
                           (bmat_t[1], bmat_in[1]), (bmat_t[2], bmat_in[2]),
                           (wout_t, wout_in), (bout_t, bout_in),
                           (iota_t, iota_in), (ident_t, ident_in),
                           (sel_t, sel_in), (ones_t, ones_in)]:
                nc.sync.dma_start(t_[:], s_[:])
            zero_t = cpool.tile([128, 1], dt.float32)
            nc.vector.memset(lnlam_t[:], LN_LAM)
            nc.vector.memset(onebias_t[:], 1.0)
            nc.vector.memset(zero_t[:], 0.0)

            # ===== phase 0: h0T = W_field^T @ xT (+ b_field) =====
            with tc.tile_pool(name="p0", bufs=2) as p0, \
                 tc.tile_pool(name="ps0", bufs=2, space="PSUM") as ps0:
                xt = p0.tile([IN_DIM, SHARD], dt.float32)
                nc.sync.dma_start(xt[:], xT_in[:])
                for t0 in range(0, SHARD, 512):
                    tl = min(512, SHARD - t0)
                    ps = ps0.tile([H, 512], dt.float32, tag="h0ps")
                    nc.tensor.matmul(ps[:, :tl], wfield_t[:], xt[:, t0:t0 + tl],
                                     start=True, stop=False)
                    nc.tensor.matmul(ps[:, :tl], bfield_t[:], ones_t[:, :tl],
                                     start=False, stop=True)
                    nc.scalar.copy(hT[0][:, t0:t0 + tl], ps[:, :tl])

            # table write + AG helper
            def build_table(l, srcT):
                with tc.tile_pool(name=f"tb{l}", bufs=2) as tp, \
                     tc.tile_pool(name=f"tps{l}", bufs=2, space="PSUM") as tps:
                    for t in range(NW):
                        ps = tps.tile([128, H], dt.float32, tag="trps")
                        nc.tensor.transpose(
                            ps[:], srcT[:, t * GRP:(t + 1) * GRP], ident_t[:H, :H])
                        tr = tp.tile([128, 128], dt.float32, tag="tr")
                        nc.scalar.copy(tr[:, :H], ps[:])
                        nc.vector.memset(tr[:, H:H + 1], 1.0)
                        nc.vector.memset(tr[:, H + 1:], 0.0)
                        nc.sync.dma_start(
                            slab[l][t * GRP:(t + 1) * GRP, :], tr[:])
                nc.gpsimd.collective_compute(
                    "AllGather", mybir.AluOpType.bypass,
                    ins=[slab[l][:]], outs=[table[l][:]],
                    replica_groups=[list(range(NCORES))])

            build_table(0, hT[0])

            # ===== layers =====
            for l in (1, 2):
                tab = table[l - 1]
                hprev = hT[l - 1]
                hacc = hT[l]
                with tc.tile_pool(name=f"pl{l}", bufs=2) as pl, \
                     tc.tile_pool(name=f"pg{l}", bufs=2) as pg, \
                     tc.tile_pool(name=f"gps{l}", bufs=2, space="PSUM") as pgs, \
                     tc.tile_pool(name=f"eps{l}", bufs=2, space="PSUM") as pes:
                    for a in (0, 1):
                        tab_half = tab[a * HALF:(a + 1) * HALF, :]
                        for w in range(NW):
                            git = pl.tile([128, WSZ // 16], dt.int16, tag="gi")
                            nc.sync.dma_start(git[:], gi_in[a][w])
                            twt = pl.tile([128, NCH], dt.float32, tag="tw")
                            nc.sync.dma_start(twt[:], tw_in[a][w])
                            xg = pg.tile([128, NCH, 128], dt.float32, tag="xg")
                            nc.gpsimd.dma_gather(
                                xg[:], tab_half, git[:], WSZ, WSZ, 128,
                                single_packet=False,
                                queue_num=(a * NW + w) % 4)
                            # w = exp(-beta*t + ln(lam)); pads t=1e9 -> 0
                            wv = pl.tile([128, NCH], dt.float32, tag="wv")
                            nc.scalar.activation(
                                wv[:], twt[:], mybir.ActivationFunctionType.Exp,
                                bias=lnlam_t[:], scale=-bet)
                            xw = pg.tile([128, NCH, AUG], dt.float32, tag="xw")
                            a_, b_ = bass.broadcast_tensor_aps(
                                xg[:, :, 0:AUG],
                                wv[:].rearrange("p (c o) -> p c o", o=1))
                            nc.vector.tensor_tensor(
                                xw[:], a_, b_, mybir.AluOpType.mult)
                            ohs = []
                            for p in range(MP):
                                rt = pl.tile([128, NCH], dt.float32,
                                             tag=f"r{p}")
                                nc.sync.dma_start(rt[:], rid_in[(p, a)][w])
                                oh = pg.tile([128, NCH, GRP], dt.float32,
                                             tag=f"oh{p}")
                                a_, b_ = bass.broadcast_tensor_aps(
                                    rt[:].rearrange("p (c o) -> p c o", o=1),
                                    iota_t[:].rearrange("p (o i) -> p o i", o=1))
                                if p == 0:
                                    nc.vector.tensor_tensor(
                                        oh[:], a_, b_,
                                        mybir.AluOpType.is_equal)
                                else:
                                    # ACT: relu(1 - |d - iota|)
                                    nc.vector.tensor_tensor(
                                        oh[:], a_, b_,
                                        mybir.AluOpType.subtract)
                                    nc.scalar.activation(
                                        oh[:], oh[:],
                                        mybir.ActivationFunctionType.Abs,
                                        bias=zero_t[:])
                                    nc.scalar.activation(
                                        oh[:], oh[:],
                                        mybir.ActivationFunctionType.Relu,
                                        bias=onebias_t[:], scale=-1.0)
                                ohs.append(oh)
                            gps = pgs.tile([AUG, R * GRP], dt.float32, tag="gps")
                            nc.vector.memset(gps[:], 0.0)
                            for c in range(NCH):
                                for p, b in enumerate(BS[c]):
                                    nc.tensor.matmul(
                                        gps[:, b * GRP:(b + 1) * GRP],
                                        xw[:, c, :], ohs[p][:, c, :],
                                        start=False, stop=False)
                            gsb = pl.tile([AUG, R * GRP], dt.float32, tag="gsb")
                            nc.scalar.copy(gsb[:], gps[:])
                            h1ps = pes.tile([H, GRP], dt.float32, tag="h1ps")
                            for r in range(R):
                                nc.tensor.matmul(
                                    h1ps[:], waug_t[l][:, r * H:(r + 1) * H],
                                    gsb[:, r * GRP:(r + 1) * GRP],
                                    start=(r == 0), stop=False)
                            hw = hprev[:, w * GRP:(w + 1) * GRP]
                            for r in range(R):
                                cm = pes.tile([H, GRP], dt.float32, tag="cm")
                                nc.tensor.matmul(
                                    cm[:], sel_t[:],
                                    gsb[:, r * GRP:(r + 1) * GRP],
                                    start=True, stop=True)
                                tmp = pl.tile([H, GRP], dt.float32, tag="tmp")
                                nc.vector.tensor_tensor(
                                    tmp[:], hw, cm[:], mybir.AluOpType.mult)
                                nc.tensor.matmul(
                                    h1ps[:], bmat_t[l][:, r * H:(r + 1) * H],
                                    tmp[:], start=False, stop=(r == R - 1))
                            dstw = hacc[:, w * GRP:(w + 1) * GRP]
                            if a == 0:
                                nc.vector.tensor_copy(dstw, h1ps[:])
                            else:
                                nc.vector.tensor_tensor(
                                    dstw, dstw, h1ps[:], mybir.AluOpType.add)
                if l == 1:
                    build_table(1, hT[1])

            # ===== output =====
            with tc.tile_pool(name="po", bufs=2) as po, \
                 tc.tile_pool(name="pso", bufs=2, space="PSUM") as pso:
                for t0 in range(0, SHARD, 512):
                    tl = min(512, SHARD - t0)
                    br = []
                    for li in range(3):
                        ps = pso.tile([OUT, 512], dt.float32, tag=f"ops{li}")
                        nc.tensor.matmul(
                            ps[:, :tl], wout_t[:, li * OUT:(li + 1) * OUT],
                            hT[li][:, t0:t0 + tl], start=True, stop=True)
                        sb = po.tile([OUT, 512], dt.float32, tag=f"osb{li}")
                        nc.scalar.activation(
                            sb[:, :tl], ps[:, :tl],
                            mybir.ActivationFunctionType.Lrelu,
                            bias=bout_t[:, li:li + 1], alpha=LRELU_SLOPE)
                        br.append(sb)
                    nc.vector.tensor_tensor(br[0][:, :tl], br[0][:, :tl],
                                            br[1][:, :tl], mybir.AluOpType.add)
                    nc.vector.tensor_tensor(br[0][:, :tl], br[0][:, :tl],
                                            br[2][:, :tl], mybir.AluOpType.add)
                    nc.sync.dma_start(outT[:, t0:t0 + tl], br[0][:, :tl])

    nc.compile()

    shared = {"wfield": wfield_np, "bfield": bfield_np,
              "waug1": waug1_np, "waug2": waug2_np,
              "bmat1": bmat1_np, "bmat2": bmat2_np,
              "wout": wout_np, "bout": bout_np, "iota": iota_np,
              "ident": ident_np, "sel": sel_np, "ones": ones_np}
    in_maps = []
    for k in range(NCORES):
        m = dict(shared)
        m["xT"] = np.ascontiguousarray(
            x_slot[k * SHARD:(k + 1) * SHARD].T)
        for key, arr in edge_inputs[k].items():
            m[key] = arr
        in_maps.append(m)

    res = run_bass_kernel_spmd(nc, in_maps, core_ids=list(range(NCORES)))
    global LAST_EXEC_NS
    LAST_EXEC_NS = res.exec_time_ns

    out_slot = np.concatenate(
        [res.results[k]["outT"].T for k in range(NCORES)], axis=0)  # [NPAD, OUT]
    return np.ascontiguousarray(out_slot[slot])



# revision 4
# speedup vs baseline: 1.9011x; 1.9011x over previous
"""Trainium2 Bass kernel for MultiRelationGNN (2-layer relational message
passing, N=50000 nodes, E=1.6M edges, H=64, R=8 relations) on 8 NeuronCores.

Strategy (edge-parallel by destination shard, fp16 data path):
  - Nodes are renumbered (degree-balanced snake) into 8 shards x 49 windows of
    128 slots. Each core owns the edges whose dst falls in its shard.
  - Per layer: every core gathers h[src] rows (fp16 256B rows [h|1|pad]) from a
    replicated DRAM table via dma_gather (4-way queue-concurrent desc-gen),
    builds one dst one-hot per 128-edge chunk (is_equal vs iota), and forms
    per-relation-pass operands xw_p = xg * wmask_p where wmask_p is the
    host-precomputed w=lam*exp(-beta*t) masked to pass p's relation block.
  - Segment-sum into per-(window, relation) aggregates G via fp16 PE matmuls,
    accumulated in one PSUM tile across both src-half arrays.
  - Aggregate-transform once per window: h1 = sum_r(A_r^T G_r + c_r*b_r
    + B_r^T (c_r .* h0)); no per-edge second matmul.
  - Tables are rebuilt (PE transposes) and AllGathered between layers.
  - Output: out = sum_l lrelu(h_l @ W_out_l + b_out_l) over own shard,
    assembled host-side through the inverse node permutation.
"""
import numpy as np

NCORES = 8
GRP = 128          # window width (nodes per window)
H = 64
AUG = 65
R = 8
LRELU_SLOPE = 0.01
LAST_EXEC_NS = None


def _wrap16(ix):
    """idx vector (mult of 16) -> [128, n/16] int16, replicated per 16-part group."""
    n = len(ix)
    a = ix.astype(np.int16).reshape(n // 16, 16).T
    return np.tile(a, (8, 1))


def _wrap128(v):
    """value vector (mult of 128) -> [128, n/128] (edge i -> [i%128, i//128])."""
    n = len(v)
    return np.ascontiguousarray(v.reshape(n // 128, 128).T)


def _host_prep(src_slot, dst_slot, rel, wgt, nwin, shard, half):
    """Per (core, array): sorted/padded edge streams + per-pass weight masks.

    Chunk c of every window gets a core-invariant list of rel-block bases
    BS[c] (union over cores/windows of blocks hit); pass p covers edges
    whose relation equals BS[c][p].
    """
    core = dst_slot // shard
    halfsel = (src_slot >= half).astype(np.int64)

    percore = []
    wsz = 1
    for k in range(NCORES):
        mk = core == k
        data = [src_slot[mk], dst_slot[mk] % shard, rel[mk], wgt[mk], halfsel[mk]]
        for a in (0, 1):
            ma = data[4] == a
            wv = data[1][ma] // GRP
            cnts = np.bincount(wv, minlength=nwin)
            wsz = max(wsz, int(cnts.max()))
        entry = {}
        for a in (0, 1):
            ma = data[4] == a
            s_a, d_a, r_a, w_a = (data[0][ma], data[1][ma], data[2][ma],
                                  data[3][ma])
            win_a = d_a // GRP
            o = np.lexsort((s_a, r_a, win_a))
            entry[a] = tuple(x[o] for x in (s_a, d_a, r_a, w_a, win_a))
        percore.append(entry)
    WSZ = ((wsz + 127) // 128) * 128
    NCH = WSZ // 128

    # sweep 1: block sets per chunk index (union over cores/arrays/windows)
    bsets = [set() for _ in range(NCH)]
    for k in range(NCORES):
        for a in (0, 1):
            s_a, d_a, r_a, w_a, win_a = percore[k][a]
            wstart = np.searchsorted(win_a, np.arange(nwin + 1))
            for w in range(nwin):
                lo, hi = int(wstart[w]), int(wstart[w + 1])
                rr = r_a[lo:hi]
                for c in range((hi - lo + 127) // 128):
                    blk = np.unique(rr[c * 128:min((c + 1) * 128, hi - lo)])
                    bsets[c].update(int(b) for b in blk)
    BS = [sorted(bs) if bs else [0] for bs in bsets]
    MP = max(len(b) for b in BS)

    inputs = []
    for k in range(NCORES):
        d = {}
        for a in (0, 1):
            s_a, d_a, r_a, w_a, win_a = percore[k][a]
            gi = np.zeros(nwin * WSZ, np.int64)
            dc = np.full(nwin * WSZ, -1.0, np.float32)
            wm = np.zeros((MP, nwin * WSZ), np.float32)
            wstart = np.searchsorted(win_a, np.arange(nwin + 1))
            for w in range(nwin):
                lo, hi = int(wstart[w]), int(wstart[w + 1])
                n = hi - lo
                off = w * WSZ
                gi[off:off + n] = s_a[lo:hi] - a * half
                dc[off:off + n] = (d_a[lo:hi] % GRP).astype(np.float32)
                rr = r_a[lo:hi]
                ww = w_a[lo:hi]
                for c in range((n + 127) // 128):
                    clo, chi = c * 128, min((c + 1) * 128, n)
                    for p, b in enumerate(BS[c]):
                        m = rr[clo:chi] == b
                        seg = np.zeros(chi - clo, np.float32)
                        seg[m] = ww[clo:chi][m]
                        wm[p, off + clo:off + chi] = seg
            d[f"gi{a}"] = np.stack([_wrap16(gi[w * WSZ:(w + 1) * WSZ])
                                    for w in range(nwin)])
            d[f"dc{a}"] = np.stack([_wrap128(dc[w * WSZ:(w + 1) * WSZ])
                                    for w in range(nwin)]).astype(np.float16)
            for p in range(MP):
                d[f"wm{p}_{a}"] = np.stack(
                    [_wrap128(wm[p, w * WSZ:(w + 1) * WSZ])
                     for w in range(nwin)]).astype(np.float16)
        inputs.append(d)
    return WSZ, NCH, BS, MP, inputs


def kernel(x, edge_index, edge_type, edge_time, lambda_sym, beta,
           W_field, b_field, W_rel1, b_rel1, W_rel2, b_rel2,
           W_out0, b_out0, W_out1, b_out1, W_out2, b_out2):
    import concourse.bacc as bacc
    import concourse.tile as tile
    import concourse.bass as bass
    import concourse.mybir as mybir
    from concourse.bass_utils import run_bass_kernel_spmd

    x = np.asarray(x, np.float32)
    edge_index = np.asarray(edge_index)
    edge_type = np.asarray(edge_type)
    edge_time = np.asarray(edge_time, np.float32)
    N, IN_DIM = x.shape
    OUT = np.asarray(W_out0).shape[1]
    lam = float(np.asarray(lambda_sym))
    bet = float(np.asarray(beta))

    # ---- node renumbering: degree-balanced snake over all windows ----
    nwin_total = ((N + NCORES * GRP - 1) // (NCORES * GRP))  # windows per core
    NWIN = nwin_total * NCORES
    SHARD = nwin_total * GRP
    NPAD = SHARD * NCORES
    HALF = NPAD // 2
    assert HALF % 2 == 0 and HALF < 32768

    src, dst = edge_index[0].astype(np.int64), edge_index[1].astype(np.int64)
    deg = np.bincount(dst, minlength=N)
    order_nodes = np.argsort(-deg, kind="stable")
    slot = np.zeros(N, np.int64)
    cnt = np.zeros(NWIN, np.int64)
    for i, v in enumerate(order_nodes):
        rowpos, cyc = i % NWIN, i // NWIN
        g = rowpos if cyc % 2 == 0 else NWIN - 1 - rowpos
        slot[v] = g * GRP + cnt[g]
        cnt[g] += 1
    assert cnt.max() <= GRP

    # edge weights computed exactly on host
    wgt = (lam * np.exp(-bet * edge_time.astype(np.float64))).astype(np.float32)

    s_slot, d_slot = slot[src], slot[dst]
    WSZ, NCH, BS, MP, edge_inputs = _host_prep(
        s_slot, d_slot, edge_type.astype(np.int64), wgt,
        nwin_total, SHARD, HALF)
    NW = nwin_total

    # ---- weight repacks (fp16) ----
    W_rel1 = np.asarray(W_rel1, np.float32)
    W_rel2 = np.asarray(W_rel2, np.float32)
    b_rel1 = np.asarray(b_rel1, np.float32)
    b_rel2 = np.asarray(b_rel2, np.float32)

    def aug_pack(Wr, br):
        out = np.zeros((AUG, R * H), np.float32)
        for r in range(R):
            out[:H, r * H:(r + 1) * H] = Wr[r, :H, :]
            out[H, r * H:(r + 1) * H] = br[r]
        return out.astype(np.float16)

    def b_pack(Wr):
        out = np.zeros((H, R * H), np.float32)
        for r in range(R):
            out[:, r * H:(r + 1) * H] = Wr[r, H:, :]
        return out.astype(np.float16)

    waug1_np, waug2_np = aug_pack(W_rel1, b_rel1), aug_pack(W_rel2, b_rel2)
    bmat1_np, bmat2_np = b_pack(W_rel1), b_pack(W_rel2)
    wout_np = np.concatenate([np.asarray(W_out0, np.float32),
                              np.asarray(W_out1, np.float32),
                              np.asarray(W_out2, np.float32)],
                             axis=1).astype(np.float16)  # [H, 3*OUT]
    bout_np = np.stack([np.asarray(b_out0, np.float32),
                        np.asarray(b_out1, np.float32),
                        np.asarray(b_out2, np.float32)], axis=1)  # [OUT, 3] f32
    iota_np = np.tile(np.arange(GRP, dtype=np.float16), (128, 1))
    ident_np = np.eye(128, dtype=np.float16)
    sel_np = np.zeros((AUG, H), np.float16)
    sel_np[H, :] = 1.0
    bfield_np = np.asarray(b_field, np.float32).reshape(H, 1)  # bias column
    wfield_np = np.asarray(W_field, np.float32).astype(np.float16)

    # per-core xT (padded slots -> zero rows), fp16
    x_slot = np.zeros((NPAD, IN_DIM), np.float32)
    x_slot[slot] = x
    x_slot = x_slot.astype(np.float16)

    # ---- build program ----
    nc = bacc.Bacc("TRN2", target_bir_lowering=False, debug=False,
                   enable_asserts=False, num_devices=NCORES,
                   num_swdge_queues=4)
    dt = mybir.dt

    def inp(name, shape, d=dt.float32):
        return nc.dram_tensor(name, shape, d, kind="ExternalInput").ap()

    xT_in = inp("xT", [IN_DIM, SHARD], dt.float16)
    gi_in = {a: inp(f"gi{a}", [NW, 128, WSZ // 16], dt.int16) for a in (0, 1)}
    dc_in = {a: inp(f"dc{a}", [NW, 128, NCH], dt.float16) for a in (0, 1)}
    wm_in = {(p, a): inp(f"wm{p}_{a}", [NW, 128, NCH], dt.float16)
             for p in range(MP) for a in (0, 1)}
    wfield_in = inp("wfield", [IN_DIM, H], dt.float16)
    bfield_in = inp("bfield", [H, 1])
    waug_in = {1: inp("waug1", [AUG, R * H], dt.float16),
               2: inp("waug2", [AUG, R * H], dt.float16)}
    bmat_in = {1: inp("bmat1", [H, R * H], dt.float16),
               2: inp("bmat2", [H, R * H], dt.float16)}
    wout_in = inp("wout", [H, 3 * OUT], dt.float16)
    bout_in = inp("bout", [OUT, 3])
    iota_in = inp("iota", [128, GRP], dt.float16)
    ident_in = inp("ident", [128, 128], dt.float16)
    sel_in = inp("sel", [AUG, H], dt.float16)
    outT = nc.dram_tensor("outT", [OUT, SHARD], dt.float32,
                          kind="ExternalOutput").ap()

    # internal DRAM: per-layer tables + AG slabs (fp16 rows of 128)
    slab = {l: nc.dram_tensor(f"slab{l}", [SHARD, 128], dt.float16,
                              kind="Internal").ap() for l in (0, 1)}
    table = {l: nc.dram_tensor(f"table{l}", [NPAD, 128], dt.float16,
                               kind="Internal", addr_space="Shared").ap()
             for l in (0, 1)}

    with tile.TileContext(nc) as tc:
        with tc.tile_pool(name="const", bufs=1) as cpool:
            wfield_t = cpool.tile([IN_DIM, H], dt.float16)
            bfield_t = cpool.tile([H, 1], dt.float32)
            waug_t = {1: cpool.tile([AUG, R * H], dt.float16, name="waug1"),
                      2: cpool.tile([AUG, R * H], dt.float16, name="waug2")}
            bmat_t = {1: cpool.tile([H, R * H], dt.float16, name="bmat1"),
                      2: cpool.tile([H, R * H], dt.float16, name="bmat2")}
            wout_t = cpool.tile([H, 3 * OUT], dt.float16)
            bout_t = cpool.tile([OUT, 3], dt.float32)
            iota_t = cpool.tile([128, GRP], dt.float16)
            ident_t = cpool.tile([128, 128], dt.float16)
            sel_t = cpool.tile([AUG, H], dt.float16)
            h0T_t = cpool.tile([H, SHARD], dt.float16)
            h1T_t = cpool.tile([H, SHARD], dt.float16)
            h2T_t = cpool.tile([H, SHARD], dt.float16)
            hT = {0: h0T_t, 1: h1T_t, 2: h2T_t}
            for t_, s_ in [(wfield_t, wfield_in), (bfield_t, bfield_in),
                           (waug_t[1], waug_in[1]), (waug_t[2], waug_in[2]),
                           (bmat_t[1], bmat_in[1]), (bmat_t[2], bmat_in[2]),
                           (wout_t, wout_in), (bout_t, bout_in),
                           (iota_t, iota_in), (ident_t, ident_in),
                           (sel_t, sel_in)]:
                nc.sync.dma_start(t_[:], s_[:])

            # ===== phase 0: h0T = W_field^T @ xT (+ b_field via ACT bias) ====
            with tc.tile_pool(name="p0", bufs=2) as p0, \
                 tc.tile_pool(name="ps0", bufs=2, space="PSUM") as ps0:
                xt = p0.tile([IN_DIM, SHARD], dt.float16)
                nc.sync.dma_start(xt[:], xT_in[:])
                for t0 in range(0, SHARD, 512):
                    tl = min(512, SHARD - t0)
                    ps = ps0.tile([H, 512], dt.float32, tag="h0ps")
                    nc.tensor.matmul(ps[:, :tl], wfield_t[:], xt[:, t0:t0 + tl],
                                     start=True, stop=True)
                    nc.scalar.activation(hT[0][:, t0:t0 + tl], ps[:, :tl],
                                         mybir.ActivationFunctionType.Identity,
                                         bias=bfield_t[:])

            # table write + AG helper
            def build_table(l, srcT):
                with tc.tile_pool(name=f"tb{l}", bufs=2) as tp, \
                     tc.tile_pool(name=f"tps{l}", bufs=2, space="PSUM") as tps:
                    for t in range(NW):
                        ps = tps.tile([128, H], dt.float16, tag="trps")
                        nc.tensor.transpose(
                            ps[:], srcT[:, t * GRP:(t + 1) * GRP], ident_t[:H, :H])
                        tr = tp.tile([128, 128], dt.float16, tag="tr")
                        nc.scalar.copy(tr[:, :H], ps[:])
                        nc.vector.memset(tr[:, H:H + 1], 1.0)
                        nc.vector.memset(tr[:, H + 1:], 0.0)
                        nc.sync.dma_start(
                            slab[l][t * GRP:(t + 1) * GRP, :], tr[:])
                nc.gpsimd.collective_compute(
                    "AllGather", mybir.AluOpType.bypass,
                    ins=[slab[l][:]], outs=[table[l][:]],
                    replica_groups=[list(range(NCORES))])

            build_table(0, hT[0])

            # ===== layers =====
            for l in (1, 2):
                tab = table[l - 1]
                hprev = hT[l - 1]
                hacc = hT[l]
                with tc.tile_pool(name=f"pl{l}", bufs=4) as pl, \
                     tc.tile_pool(name=f"pg{l}", bufs=4) as pg, \
                     tc.tile_pool(name=f"px{l}", bufs=2) as px, \
                     tc.tile_pool(name=f"gps{l}", bufs=2, space="PSUM") as pgs, \
                     tc.tile_pool(name=f"eps{l}", bufs=2, space="PSUM") as pes:
                    for w in range(NW):
                        gps = pgs.tile([AUG, R * GRP], dt.float32, tag="gps")
                        nc.vector.memset(gps[:], 0.0)
                        for a in (0, 1):
                            tab_half = tab[a * HALF:(a + 1) * HALF, :]
                            git = pl.tile([128, WSZ // 16], dt.int16, tag="gi")
                            nc.sync.dma_start(git[:], gi_in[a][w])
                            dct = pl.tile([128, NCH], dt.float16, tag="dc")
                            nc.sync.dma_start(dct[:], dc_in[a][w])
                            wmt = []
                            for p in range(MP):
                                t_ = pl.tile([128, NCH], dt.float16,
                                             tag=f"wm{p}")
                                nc.sync.dma_start(t_[:], wm_in[(p, a)][w])
                                wmt.append(t_)
                            xg = pg.tile([128, NCH, 128], dt.float16, tag="xg")
                            nc.gpsimd.dma_gather(
                                xg[:], tab_half, git[:], WSZ, WSZ, 128,
                                single_packet=False,
                                queue_num=(2 * w + a) % 4)
                            oh = px.tile([128, NCH, GRP], dt.float16, tag="oh")
                            a_, b_ = bass.broadcast_tensor_aps(
                                dct[:].rearrange("p (c o) -> p c o", o=1),
                                iota_t[:].rearrange("p (o i) -> p o i", o=1))
                            nc.vector.tensor_tensor(
                                oh[:], a_, b_, mybir.AluOpType.is_equal)
                            for p in range(MP):
                                cs = [c for c in range(NCH)
                                      if len(BS[c]) > p]
                                if not cs:
                                    continue
                                c0, c1 = min(cs), max(cs) + 1
                                xw = px.tile([128, NCH, AUG], dt.float16,
                                             tag=f"xw{p}")
                                a_, b_ = bass.broadcast_tensor_aps(
                                    xg[:, c0:c1, 0:AUG],
                                    wmt[p][:, c0:c1].rearrange(
                                        "p (c o) -> p c o", o=1))
                                nc.vector.tensor_tensor(
                                    xw[:, c0:c1, :], a_, b_,
                                    mybir.AluOpType.mult)
                                for c in cs:
                                    b = BS[c][p]
                                    nc.tensor.matmul(
                                        gps[:, b * GRP:(b + 1) * GRP],
                                        xw[:, c, :], oh[:, c, :],
                                        start=False, stop=False)
                        # ---- aggregate transform (once per window) ----
                        gsb = pl.tile([AUG, R * GRP], dt.float16, tag="gsb")
                        nc.scalar.copy(gsb[:], gps[:])
                        h1ps = pes.tile([H, GRP], dt.float32, tag="h1ps")
                        for r in range(R):
                            nc.tensor.matmul(
                                h1ps[:], waug_t[l][:, r * H:(r + 1) * H],
                                gsb[:, r * GRP:(r + 1) * GRP],
                                start=(r == 0), stop=False)
                        hw = hprev[:, w * GRP:(w + 1) * GRP]
                        for r in range(R):
                            cm = pes.tile([H, GRP], dt.float32, tag="cm")
                            nc.tensor.matmul(
                                cm[:], sel_t[:],
                                gsb[:, r * GRP:(r + 1) * GRP],
                                start=True, stop=True)
                            cmh = pl.tile([H, GRP], dt.float16, tag="cmh")
                            nc.scalar.copy(cmh[:], cm[:])
                            tmp = pl.tile([H, GRP], dt.float16, tag="tmp")
                            nc.vector.tensor_tensor(
                                tmp[:], hw, cmh[:], mybir.AluOpType.mult)
                            nc.tensor.matmul(
                                h1ps[:], bmat_t[l][:, r * H:(r + 1) * H],
                                tmp[:], start=False, stop=(r == R - 1))
                        nc.vector.tensor_copy(hacc[:, w * GRP:(w + 1) * GRP],
                                              h1ps[:])
                if l == 1:
                    build_table(1, hT[1])

            # ===== output =====
            with tc.tile_pool(name="po", bufs=2) as po, \
                 tc.tile_pool(name="pso", bufs=2, space="PSUM") as pso:
                for t0 in range(0, SHARD, 512):
                    tl = min(512, SHARD - t0)
                    br = []
                    for li in range(3):
                        ps = pso.tile([OUT, 512], dt.float32, tag=f"ops{li}")
                        nc.tensor.matmul(
                            ps[:, :tl], wout_t[:, li * OUT:(li + 1) * OUT],
                            hT[li][:, t0:t0 + tl], start=True, stop=True)
                        sb = po.tile([OUT, 512], dt.float32, tag=f"osb{li}")
                        nc.scalar.activation(
                            sb[:, :tl], ps[:, :tl],
                            mybir.ActivationFunctionType.Lrelu,
                            bias=bout_t[:, li:li + 1], alpha=LRELU_SLOPE)
                        br.append(sb)
                    nc.vector.tensor_tensor(br[0][:, :tl], br[0][:, :tl],
                                            br[1][:, :tl], mybir.AluOpType.add)
                    nc.vector.tensor_tensor(br[0][:, :tl], br[0][:, :tl],
                                            br[2][:, :tl], mybir.AluOpType.add)
                    nc.sync.dma_start(outT[:, t0:t0 + tl], br[0][:, :tl])

    nc.compile()

    shared = {"wfield": wfield_np, "bfield": bfield_np,
              "waug1": waug1_np, "waug2": waug2_np,
              "bmat1": bmat1_np, "bmat2": bmat2_np,
              "wout": wout_np, "bout": bout_np, "iota": iota_np,
              "ident": ident_np, "sel": sel_np}
    in_maps = []
    for k in range(NCORES):
        m = dict(shared)
        m["xT"] = np.ascontiguousarray(
            x_slot[k * SHARD:(k + 1) * SHARD].T)
        for key, arr in edge_inputs[k].items():
            m[key] = arr
        in_maps.append(m)

    res = run_bass_kernel_spmd(nc, in_maps, core_ids=list(range(NCORES)))
    global LAST_EXEC_NS
    LAST_EXEC_NS = res.exec_time_ns

    out_slot = np.concatenate(
        [res.results[k]["outT"].T for k in range(NCORES)], axis=0)  # [NPAD, OUT]
    return np.ascontiguousarray(out_slot[slot])


# revision 9
# speedup vs baseline: 2.1823x; 1.1479x over previous
"""Trainium2 Bass kernel for MultiRelationGNN (2-layer relational message
passing, N=50000 nodes, E=1.6M edges, H=64, R=8 relations) on 8 NeuronCores.

Strategy (edge-parallel by destination shard, fp16 data path):
  - Nodes are renumbered (degree-balanced snake) into 8 shards x 49 windows of
    128 slots. Each core owns the edges whose dst falls in its shard.
  - Per layer: every core gathers h[src] rows (fp16 256B rows [h|1|pad]) from a
    replicated DRAM table via dma_gather (4-way queue-concurrent desc-gen),
    builds one dst one-hot per 128-edge chunk (is_equal vs iota), and forms
    per-relation-pass operands xw_p = xg * wmask_p where wmask_p is the
    host-precomputed w=lam*exp(-beta*t) masked to pass p's relation block.
  - Segment-sum into per-(window, relation) aggregates G via fp16 PE matmuls,
    accumulated in one PSUM tile across both src-half arrays.
  - Aggregate-transform once per window: h1 = sum_r(A_r^T G_r + c_r*b_r
    + B_r^T (c_r .* h0)); no per-edge second matmul.
  - Tables are rebuilt (PE transposes) and AllGathered between layers.
  - Output: out = sum_l lrelu(h_l @ W_out_l + b_out_l) over own shard,
    assembled host-side through the inverse node permutation.
"""
import numpy as np

NCORES = 8
GRP = 128          # window width (nodes per window)
H = 64
AUG = 65
R = 8
LRELU_SLOPE = 0.01
LAST_EXEC_NS = None


def _wrap16(ix):
    """idx vector (mult of 16) -> [128, n/16] int16, replicated per 16-part group."""
    n = len(ix)
    a = ix.astype(np.int16).reshape(n // 16, 16).T
    return np.tile(a, (8, 1))


def _wrap128(v):
    """value vector (mult of 128) -> [128, n/128] (edge i -> [i%128, i//128])."""
    n = len(v)
    return np.ascontiguousarray(v.reshape(n // 128, 128).T)


def _host_prep(src_slot, dst_slot, rel, wgt, nwin, shard, half):
    """Per (core, array): sorted/padded edge streams + per-pass weight masks.

    Chunk c of every window gets a core-invariant list of rel-block bases
    BS[c] (union over cores/windows of blocks hit); pass p covers edges
    whose relation equals BS[c][p].
    """
    core = dst_slot // shard
    halfsel = (src_slot >= half).astype(np.int64)

    percore = []
    wsz = 1
    for k in range(NCORES):
        mk = core == k
        data = [src_slot[mk], dst_slot[mk] % shard, rel[mk], wgt[mk], halfsel[mk]]
        for a in (0, 1):
            ma = data[4] == a
            wv = data[1][ma] // GRP
            cnts = np.bincount(wv, minlength=nwin)
            wsz = max(wsz, int(cnts.max()))
        entry = {}
        for a in (0, 1):
            ma = data[4] == a
            s_a, d_a, r_a, w_a = (data[0][ma], data[1][ma], data[2][ma],
                                  data[3][ma])
            win_a = d_a // GRP
            o = np.lexsort((s_a, r_a, win_a))
            entry[a] = tuple(x[o] for x in (s_a, d_a, r_a, w_a, win_a))
        percore.append(entry)
    WSZ = ((wsz + 127) // 128) * 128
    NCH = WSZ // 128

    # sweep 1: block sets per chunk index (union over cores/arrays/windows)
    bsets = [set() for _ in range(NCH)]
    for k in range(NCORES):
        for a in (0, 1):
            s_a, d_a, r_a, w_a, win_a = percore[k][a]
            wstart = np.searchsorted(win_a, np.arange(nwin + 1))
            for w in range(nwin):
                lo, hi = int(wstart[w]), int(wstart[w + 1])
                rr = r_a[lo:hi]
                for c in range((hi - lo + 127) // 128):
                    blk = np.unique(rr[c * 128:min((c + 1) * 128, hi - lo)])
                    bsets[c].update(int(b) for b in blk)
    BS = [sorted(bs) if bs else [0] for bs in bsets]
    MP = max(len(b) for b in BS)

    inputs = []
    for k in range(NCORES):
        d = {}
        for a in (0, 1):
            s_a, d_a, r_a, w_a, win_a = percore[k][a]
            gi = np.zeros(nwin * WSZ, np.int64)
            dc = np.full(nwin * WSZ, -1.0, np.float32)
            wm = np.zeros((MP, nwin * WSZ), np.float32)
            wstart = np.searchsorted(win_a, np.arange(nwin + 1))
            for w in range(nwin):
                lo, hi = int(wstart[w]), int(wstart[w + 1])
                n = hi - lo
                off = w * WSZ
                gi[off:off + n] = s_a[lo:hi] - a * half
                dc[off:off + n] = (d_a[lo:hi] % GRP).astype(np.float32)
                rr = r_a[lo:hi]
                ww = w_a[lo:hi]
                for c in range((n + 127) // 128):
                    clo, chi = c * 128, min((c + 1) * 128, n)
                    for p, b in enumerate(BS[c]):
                        m = rr[clo:chi] == b
                        seg = np.zeros(chi - clo, np.float32)
                        seg[m] = ww[clo:chi][m]
                        wm[p, off + clo:off + chi] = seg
            d[f"gi{a}"] = np.stack([_wrap16(gi[w * WSZ:(w + 1) * WSZ])
                                    for w in range(nwin)])
            # packed per-window fp16 edge metadata: [dstcol | wm_0 | .. | wm_MP-1]
            ed = np.stack(
                [np.concatenate(
                    [_wrap128(dc[w * WSZ:(w + 1) * WSZ])] +
                    [_wrap128(wm[p, w * WSZ:(w + 1) * WSZ])
                     for p in range(MP)], axis=1)
                 for w in range(nwin)]).astype(np.float16)
            d[f"ed{a}"] = ed
        inputs.append(d)
    return WSZ, NCH, BS, MP, inputs


def kernel(x, edge_index, edge_type, edge_time, lambda_sym, beta,
           W_field, b_field, W_rel1, b_rel1, W_rel2, b_rel2,
           W_out0, b_out0, W_out1, b_out1, W_out2, b_out2):
    import concourse.bacc as bacc
    import concourse.tile as tile
    import concourse.bass as bass
    import concourse.mybir as mybir
    from concourse.bass_utils import run_bass_kernel_spmd

    x = np.asarray(x, np.float32)
    edge_index = np.asarray(edge_index)
    edge_type = np.asarray(edge_type)
    edge_time = np.asarray(edge_time, np.float32)
    N, IN_DIM = x.shape
    OUT = np.asarray(W_out0).shape[1]
    lam = float(np.asarray(lambda_sym))
    bet = float(np.asarray(beta))

    # ---- node renumbering: degree-balanced snake over all windows ----
    nwin_total = ((N + NCORES * GRP - 1) // (NCORES * GRP))  # windows per core
    NWIN = nwin_total * NCORES
    SHARD = nwin_total * GRP
    NPAD = SHARD * NCORES
    HALF = NPAD // 2
    assert HALF % 2 == 0 and HALF < 32768

    src, dst = edge_index[0].astype(np.int64), edge_index[1].astype(np.int64)
    deg = np.bincount(dst, minlength=N)
    order_nodes = np.argsort(-deg, kind="stable")
    slot = np.zeros(N, np.int64)
    cnt = np.zeros(NWIN, np.int64)
    for i, v in enumerate(order_nodes):
        rowpos, cyc = i % NWIN, i // NWIN
        g = rowpos if cyc % 2 == 0 else NWIN - 1 - rowpos
        slot[v] = g * GRP + cnt[g]
        cnt[g] += 1
    assert cnt.max() <= GRP

    # edge weights computed exactly on host
    wgt = (lam * np.exp(-bet * edge_time.astype(np.float64))).astype(np.float32)

    s_slot, d_slot = slot[src], slot[dst]
    WSZ, NCH, BS, MP, edge_inputs = _host_prep(
        s_slot, d_slot, edge_type.astype(np.int64), wgt,
        nwin_total, SHARD, HALF)
    NW = nwin_total

    # ---- weight repacks (fp16) ----
    W_rel1 = np.asarray(W_rel1, np.float32)
    W_rel2 = np.asarray(W_rel2, np.float32)
    b_rel1 = np.asarray(b_rel1, np.float32)
    b_rel2 = np.asarray(b_rel2, np.float32)

    def aug_pack(Wr, br):
        out = np.zeros((AUG, R * H), np.float32)
        for r in range(R):
            out[:H, r * H:(r + 1) * H] = Wr[r, :H, :]
            out[H, r * H:(r + 1) * H] = br[r]
        return out.astype(np.float16)

    def b_pack(Wr):
        # paired stacking for K=128 matmuls: [B_2j ; B_2j+1] -> [128, (R/2)*H]
        out = np.zeros((2 * H, (R // 2) * H), np.float32)
        for j in range(R // 2):
            out[:H, j * H:(j + 1) * H] = Wr[2 * j, H:, :]
            out[H:, j * H:(j + 1) * H] = Wr[2 * j + 1, H:, :]
        return out.astype(np.float16)

    waug1_np, waug2_np = aug_pack(W_rel1, b_rel1), aug_pack(W_rel2, b_rel2)
    bmat1_np, bmat2_np = b_pack(W_rel1), b_pack(W_rel2)
    wout_np = np.concatenate([np.asarray(W_out0, np.float32),
                              np.asarray(W_out1, np.float32),
                              np.asarray(W_out2, np.float32)],
                             axis=1).astype(np.float16)  # [H, 3*OUT]
    bout_np = np.stack([np.asarray(b_out0, np.float32),
                        np.asarray(b_out1, np.float32),
                        np.asarray(b_out2, np.float32)], axis=1)  # [OUT, 3] f32
    iota_np = np.tile(np.arange(GRP, dtype=np.float16), (128, 1))
    ident_np = np.eye(128, dtype=np.float16)
    sel_np = np.zeros((AUG, H), np.float16)
    sel_np[H, :] = 1.0
    bfield_np = np.asarray(b_field, np.float32).reshape(H, 1)  # bias column
    wfield_np = np.asarray(W_field, np.float32).astype(np.float16)

    # per-core xT (padded slots -> zero rows), fp16
    x_slot = np.zeros((NPAD, IN_DIM), np.float32)
    x_slot[slot] = x
    x_slot = x_slot.astype(np.float16)

    # ---- build program ----
    nc = bacc.Bacc("TRN2", target_bir_lowering=False, debug=False,
                   enable_asserts=False, num_devices=NCORES,
                   num_swdge_queues=4)
    dt = mybir.dt

    def inp(name, shape, d=dt.float32):
        return nc.dram_tensor(name, shape, d, kind="ExternalInput").ap()

    xT_in = inp("xT", [IN_DIM, SHARD], dt.float16)
    gi_in = {a: inp(f"gi{a}", [NW, 128, WSZ // 16], dt.int16) for a in (0, 1)}
    ed_in = {a: inp(f"ed{a}", [NW, 128, (1 + MP) * NCH], dt.float16)
             for a in (0, 1)}
    wfield_in = inp("wfield", [IN_DIM, H], dt.float16)
    bfield_in = inp("bfield", [H, 1])
    waug_in = {1: inp("waug1", [AUG, R * H], dt.float16),
               2: inp("waug2", [AUG, R * H], dt.float16)}
    bmat_in = {1: inp("bmat1", [2 * H, (R // 2) * H], dt.float16),
               2: inp("bmat2", [2 * H, (R // 2) * H], dt.float16)}
    wout_in = inp("wout", [H, 3 * OUT], dt.float16)
    bout_in = inp("bout", [OUT, 3])
    iota_in = inp("iota", [128, GRP], dt.float16)
    ident_in = inp("ident", [128, 128], dt.float16)
    sel_in = inp("sel", [AUG, H], dt.float16)
    outT = nc.dram_tensor("outT", [OUT, SHARD], dt.float32,
                          kind="ExternalOutput").ap()

    # internal DRAM: per-layer tables + AG slabs (fp16 rows of 128)
    slab = {l: nc.dram_tensor(f"slab{l}", [SHARD, 128], dt.float16,
                              kind="Internal").ap() for l in (0, 1)}
    table = {l: nc.dram_tensor(f"table{l}", [NPAD, 128], dt.float16,
                               kind="Internal", addr_space="Shared").ap()
             for l in (0, 1)}

    with tile.TileContext(nc) as tc:
        with tc.tile_pool(name="const", bufs=1) as cpool:
            wfield_t = cpool.tile([IN_DIM, H], dt.float16)
            bfield_t = cpool.tile([H, 1], dt.float32)
            waug_t = {1: cpool.tile([AUG, R * H], dt.float16, name="waug1"),
                      2: cpool.tile([AUG, R * H], dt.float16, name="waug2")}
            bmat_t = {1: cpool.tile([2 * H, (R // 2) * H], dt.float16,
                                    name="bmat1"),
                      2: cpool.tile([2 * H, (R // 2) * H], dt.float16,
                                    name="bmat2")}
            wout_t = cpool.tile([H, 3 * OUT], dt.float16)
            bout_t = cpool.tile([OUT, 3], dt.float32)
            iota_t = cpool.tile([128, GRP], dt.float16)
            ident_t = cpool.tile([128, 128], dt.float16)
            sel_t = cpool.tile([AUG, H], dt.float16)
            h0T_t = cpool.tile([H, SHARD], dt.float16)
            h1T_t = cpool.tile([H, SHARD], dt.float16)
            h2T_t = cpool.tile([H, SHARD], dt.float16)
            hT = {0: h0T_t, 1: h1T_t, 2: h2T_t}
            for t_, s_ in [(wfield_t, wfield_in), (bfield_t, bfield_in),
                           (waug_t[1], waug_in[1]), (waug_t[2], waug_in[2]),
                           (bmat_t[1], bmat_in[1]), (bmat_t[2], bmat_in[2]),
                           (wout_t, wout_in), (bout_t, bout_in),
                           (iota_t, iota_in), (ident_t, ident_in),
                           (sel_t, sel_in)]:
                nc.sync.dma_start(t_[:], s_[:])

            # ===== phase 0: h0T = W_field^T @ xT (+ b_field via ACT bias) ====
            with tc.tile_pool(name="p0", bufs=2) as p0, \
                 tc.tile_pool(name="ps0", bufs=2, space="PSUM") as ps0:
                xt = p0.tile([IN_DIM, SHARD], dt.float16)
                nc.sync.dma_start(xt[:], xT_in[:])
                for t0 in range(0, SHARD, 512):
                    tl = min(512, SHARD - t0)
                    ps = ps0.tile([H, 512], dt.float32, tag="h0ps")
                    nc.tensor.matmul(ps[:, :tl], wfield_t[:], xt[:, t0:t0 + tl],
                                     start=True, stop=True)
                    nc.scalar.activation(hT[0][:, t0:t0 + tl], ps[:, :tl],
                                         mybir.ActivationFunctionType.Identity,
                                         bias=bfield_t[:])

            # table write + AG helper
            def build_table(l, srcT):
                with tc.tile_pool(name=f"tb{l}", bufs=2) as tp, \
                     tc.tile_pool(name=f"tps{l}", bufs=2, space="PSUM") as tps:
                    for t in range(NW):
                        ps = tps.tile([128, H], dt.float16, tag="trps")
                        nc.tensor.transpose(
                            ps[:], srcT[:, t * GRP:(t + 1) * GRP], ident_t[:H, :H])
                        tr = tp.tile([128, 128], dt.float16, tag="tr")
                        nc.scalar.copy(tr[:, :H], ps[:])
                        nc.vector.memset(tr[:, H:H + 1], 1.0)
                        nc.vector.memset(tr[:, H + 1:], 0.0)
                        nc.sync.dma_start(
                            slab[l][t * GRP:(t + 1) * GRP, :], tr[:])
                nc.gpsimd.collective_compute(
                    "AllGather", mybir.AluOpType.bypass,
                    ins=[slab[l][:]], outs=[table[l][:]],
                    replica_groups=[list(range(NCORES))])

            build_table(0, hT[0])

            # ===== layers =====
            for l in (1, 2):
                tab = table[l - 1]
                hprev = hT[l - 1]
                hacc = hT[l]
                with tc.tile_pool(name=f"pl{l}", bufs=8) as pl, \
                     tc.tile_pool(name=f"pg{l}", bufs=8) as pg, \
                     tc.tile_pool(name=f"px{l}", bufs=4) as px, \
                     tc.tile_pool(name=f"pt{l}", bufs=2) as pt, \
                     tc.tile_pool(name=f"gps{l}", bufs=2, space="PSUM") as pgs, \
                     tc.tile_pool(name=f"eps{l}", bufs=2, space="PSUM") as pes, \
                     tc.tile_pool(name=f"cps{l}", bufs=1, space="PSUM") as pcm, \
                     tc.tile_pool(name=f"tps{l}", bufs=1, space="PSUM") as ptp:
                    for w in range(NW):
                        gps = pgs.tile([AUG, R * GRP], dt.float32, tag="gps")
                        nc.vector.memset(gps[:], 0.0)
                        for a in (0, 1):
                            tab_half = tab[a * HALF:(a + 1) * HALF, :]
                            git = pl.tile([128, WSZ // 16], dt.int16, tag="gi")
                            nc.sync.dma_start(git[:], gi_in[a][w])
                            edt = pl.tile([128, (1 + MP) * NCH], dt.float16,
                                          tag="ed")
                            nc.sync.dma_start(edt[:], ed_in[a][w])
                            dct = edt[:, 0:NCH]
                            xg = pg.tile([128, NCH, 128], dt.float16, tag="xg")
                            nc.gpsimd.dma_gather(
                                xg[:], tab_half, git[:], WSZ, WSZ, 128,
                                single_packet=False,
                                queue_num=(2 * w + a) % 4)
                            oh = px.tile([128, NCH, GRP], dt.float16, tag="oh")
                            a_, b_ = bass.broadcast_tensor_aps(
                                dct.rearrange("p (c o) -> p c o", o=1),
                                iota_t[:].rearrange("p (o i) -> p o i", o=1))
                            nc.vector.tensor_tensor(
                                oh[:], a_, b_, mybir.AluOpType.is_equal)
                            for p in range(MP):
                                cs = [c for c in range(NCH)
                                      if len(BS[c]) > p]
                                if not cs:
                                    continue
                                c0, c1 = min(cs), max(cs) + 1
                                wmp = edt[:, (1 + p) * NCH:(2 + p) * NCH]
                                xw = px.tile([128, NCH, AUG], dt.float16,
                                             tag=f"xw{p}")
                                a_, b_ = bass.broadcast_tensor_aps(
                                    xg[:, c0:c1, 0:AUG],
                                    wmp[:, c0:c1].rearrange(
                                        "p (c o) -> p c o", o=1))
                                nc.vector.tensor_tensor(
                                    xw[:, c0:c1, :], a_, b_,
                                    mybir.AluOpType.mult)
                                for c in cs:
                                    b = BS[c][p]
                                    nc.tensor.matmul(
                                        gps[:, b * GRP:(b + 1) * GRP],
                                        xw[:, c, :], oh[:, c, :],
                                        start=False, stop=False)
                        # ---- aggregate transform (once per window) ----
                        gsb = pl.tile([AUG, R * GRP], dt.float16, tag="gsb")
                        nc.scalar.copy(gsb[:], gps[:])
                        gv = gsb[:].rearrange("p (j t d) -> p j t d",
                                              j=R // 2, t=2, d=GRP)
                        h1ps = pes.tile([H, GRP], dt.float32, tag="h1ps")
                        for r in range(R):
                            nc.tensor.matmul(
                                h1ps[:], waug_t[l][:, r * H:(r + 1) * H],
                                gsb[:, r * GRP:(r + 1) * GRP],
                                start=(r == 0), stop=False)
                        hw = hprev[:, w * GRP:(w + 1) * GRP]
                        hw2 = pl.tile([2 * H, GRP], dt.float16, tag="hw2")
                        nc.scalar.copy(hw2[:H, :], hw)
                        nc.scalar.copy(hw2[H:, :], hw)
                        cmp_ = pcm.tile([2 * H, (R // 2) * GRP], dt.float32,
                                        tag="cmps")
                        nc.tensor.matmul(cmp_[:H, :], sel_t[:],
                                         gv[:, :, 0, :], start=True, stop=True)
                        nc.tensor.matmul(cmp_[H:, :], sel_t[:],
                                         gv[:, :, 1, :], start=True, stop=True)
                        cmh = pl.tile([2 * H, (R // 2) * GRP], dt.float16,
                                      tag="cmh")
                        nc.scalar.copy(cmh[:], cmp_[:])
                        tmp = pl.tile([2 * H, (R // 2) * GRP], dt.float16,
                                      tag="tmp")
                        a_, b_ = bass.broadcast_tensor_aps(
                            cmh[:].rearrange("p (j d) -> p j d", d=GRP),
                            hw2[:].rearrange("p (o d) -> p o d", o=1))
                        nc.vector.tensor_tensor(
                            tmp[:], a_, b_, mybir.AluOpType.mult)
                        for j in range(R // 2):
                            nc.tensor.matmul(
                                h1ps[:], bmat_t[l][:, j * H:(j + 1) * H],
                                tmp[:, j * GRP:(j + 1) * GRP],
                                start=False, stop=(j == R // 2 - 1))
                        nc.vector.tensor_copy(hacc[:, w * GRP:(w + 1) * GRP],
                                              h1ps[:])
                        if l == 1:
                            # build next-layer table slab inline (overlapped)
                            ps = ptp.tile([128, H], dt.float16, tag="trps")
                            nc.tensor.transpose(
                                ps[:], hacc[:, w * GRP:(w + 1) * GRP],
                                ident_t[:H, :H])
                            tr = pt.tile([128, 128], dt.float16, tag="tr")
                            nc.scalar.copy(tr[:, :H], ps[:])
                            nc.vector.memset(tr[:, H:H + 1], 1.0)
                            nc.vector.memset(tr[:, H + 1:], 0.0)
                            nc.sync.dma_start(
                                slab[1][w * GRP:(w + 1) * GRP, :], tr[:])
                if l == 1:
                    nc.gpsimd.collective_compute(
                        "AllGather", mybir.AluOpType.bypass,
                        ins=[slab[1][:]], outs=[table[1][:]],
                        replica_groups=[list(range(NCORES))])

            # ===== output =====
            with tc.tile_pool(name="po", bufs=2) as po, \
                 tc.tile_pool(name="pso", bufs=2, space="PSUM") as pso:
                for t0 in range(0, SHARD, 512):
                    tl = min(512, SHARD - t0)
                    br = []
                    for li in range(3):
                        ps = pso.tile([OUT, 512], dt.float32, tag=f"ops{li}")
                        nc.tensor.matmul(
                            ps[:, :tl], wout_t[:, li * OUT:(li + 1) * OUT],
                            hT[li][:, t0:t0 + tl], start=True, stop=True)
                        sb = po.tile([OUT, 512], dt.float32, tag=f"osb{li}")
                        nc.scalar.activation(
                            sb[:, :tl], ps[:, :tl],
                            mybir.ActivationFunctionType.Lrelu,
                            bias=bout_t[:, li:li + 1], alpha=LRELU_SLOPE)
                        br.append(sb)
                    nc.vector.tensor_tensor(br[0][:, :tl], br[0][:, :tl],
                                            br[1][:, :tl], mybir.AluOpType.add)
                    nc.vector.tensor_tensor(br[0][:, :tl], br[0][:, :tl],
                                            br[2][:, :tl], mybir.AluOpType.add)
                    nc.sync.dma_start(outT[:, t0:t0 + tl], br[0][:, :tl])

    nc.compile()

    shared = {"wfield": wfield_np, "bfield": bfield_np,
              "waug1": waug1_np, "waug2": waug2_np,
              "bmat1": bmat1_np, "bmat2": bmat2_np,
              "wout": wout_np, "bout": bout_np, "iota": iota_np,
              "ident": ident_np, "sel": sel_np}
    in_maps = []
    for k in range(NCORES):
        m = dict(shared)
        m["xT"] = np.ascontiguousarray(
            x_slot[k * SHARD:(k + 1) * SHARD].T)
        for key, arr in edge_inputs[k].items():
            m[key] = arr
        in_maps.append(m)

    res = run_bass_kernel_spmd(nc, in_maps, core_ids=list(range(NCORES)))
    global LAST_EXEC_NS
    LAST_EXEC_NS = res.exec_time_ns

    out_slot = np.concatenate(
        [res.results[k]["outT"].T for k in range(NCORES)], axis=0)  # [NPAD, OUT]
    return np.ascontiguousarray(out_slot[slot])
